# revision 3
# baseline (speedup 1.0000x reference)
"""BiLSTM (B=32, S=512, I=H=1024) Trainium2 kernel over 8 NeuronCores.

Strategy: tensor-parallel over the gate dimension (each core owns a 128-row
H-slice and its four gate blocks), both directions on all 8 cores as two
staggered per-direction chains:

  - per-direction PSUM gate banks (walrus requires matmul dst partition 0);
    gate order [i|f|g|o] with true tanh on g: per dir 4 ACT + 4 DVE ops.
  - per-step, per-direction AllGather of the PE-transposed h-slice; the
    emission order is arranged so the PE queue is [mm_f x8, mm_b x8,
    xproj-filler, tp_f, tp_b] -- the transposes sit AFTER both matmul
    groups, so direction b's matmuls are not head-of-line blocked behind
    a transpose that waits on direction f's elementwise chain.  This lets
    the two directions' exchange latencies overlap with each other's
    compute (the previous version serialized them: 2x10.7us per step).
  - xp inject via an identity matmul (start=True) pre-runs during the
    exchange window.
  - x-projection chunks are interleaved through the recurrence loop (one
    per 4 steps per direction, consumed from opposite sequence ends) to
    hide their PE time in exchange gaps and keep the PE HAM-warm.
  - output written per-step as (S, 64, 128) [fwd rows 0:32 at t, bwd rows
    32:64 belong to time S-1-t] and un-reversed on the host.
"""

S_FIXED = 512
LAST_EXEC_NS = None

import numpy as np

import concourse.bass as bass
import concourse.bacc as bacc
import concourse.mybir as mybir
import concourse.tile as tile
from concourse.tile_rust import add_dep_helper

# The axon client has no /dev/neuron*, so the driver's NC/routing maps are
# unavailable.  A plausible identity map is fine for client-side validation.
import concourse.libnrt as _libnrt

try:
    _libnrt.get_trn2_nc_mapping()
except Exception:
    _libnrt.get_trn2_nc_mapping = lambda: {(0, i): i for i in range(8)}
try:
    _libnrt.get_device_id_to_routing_id_mapping()
except Exception:
    _fake_rid_map = lambda: {i: i for i in range(16)}
    _libnrt.get_device_id_to_routing_id_mapping = _fake_rid_map
    import concourse.bass_interp as _bi
    import concourse.replica_groups as _rg

    _bi.get_device_id_to_routing_id_mapping = _fake_rid_map
    _rg.get_device_id_to_routing_id_mapping = _fake_rid_map

P = 128
B = 32
B2 = 2 * B
I_DIM = 1024
H_DIM = 1024
NCORES = 8
KCH = H_DIM // P          # 8 k-chunks of the hidden dim
GS = 4 * H_DIM // NCORES  # 512 gate rows per core, order [i|f|g|o]
F32 = mybir.dt.float32
F32R = mybir.dt.float32r
SIG = mybir.ActivationFunctionType.Sigmoid
TANH = mybir.ActivationFunctionType.Tanh


def host_prep(x, W_ii, W_hi, b_i, W_ii_r, W_hi_r, b_i_r, S):
    """Build the 8 per-core input maps."""
    x = np.asarray(x, np.float32)
    # xT[i, s*B+b] = x[b, s, i]
    xT = np.ascontiguousarray(x.transpose(2, 1, 0).reshape(I_DIM, S * B))

    def slices(W, bvec, core):
        # gate rows for this core, in-slice order [i|f|g|o]
        rows_i = np.arange(core * P, core * P + P)
        rows = np.concatenate(
            [rows_i, H_DIM + rows_i, 2 * H_DIM + rows_i, 3 * H_DIM + rows_i]
        )
        Ws = W[rows, :].astype(np.float32)
        bs = bvec[rows].astype(np.float32)
        return np.ascontiguousarray(Ws.T), bs.reshape(1, GS).copy()

    id32 = np.eye(B, dtype=np.float32)
    ones128 = np.ones((1, P), dtype=np.float32)
    in_maps = []
    for c in range(NCORES):
        wiT_f, bias_f = slices(np.asarray(W_ii), np.asarray(b_i), c)
        whT_f, _ = slices(np.asarray(W_hi), np.asarray(b_i), c)
        wiT_b, bias_b = slices(np.asarray(W_ii_r), np.asarray(b_i_r), c)
        whT_b, _ = slices(np.asarray(W_hi_r), np.asarray(b_i_r), c)
        in_maps.append({
            "xT": xT,
            "wiT_f": wiT_f, "whT_f": whT_f, "bias_f": bias_f,
            "wiT_b": wiT_b, "whT_b": whT_b, "bias_b": bias_b,
            "id32": id32, "id32f": id32, "ones128": ones128,
        })
    return in_maps


def host_assemble(results, S):
    """results[c]["out"]: (S, 64, P) -> full (B, S, 2H)."""
    out = np.empty((B, S, 2 * H_DIM), np.float32)
    for c in range(NCORES):
        o = results[c]["out"]  # (S, 64, P); rows 0:32 fwd@t, 32:64 bwd@S-1-t
        out[:, :, c * P:(c + 1) * P] = o[:, 0:B, :].transpose(1, 0, 2)
        out[:, :, H_DIM + c * P:H_DIM + (c + 1) * P] = (
            o[::-1, B:B2, :].transpose(1, 0, 2)
        )
    return out


def build_kernel(S, interleave_xproj=True):
    nc = bacc.Bacc(None)
    SB = S * B
    MCH = SB // P  # sb-chunks of 128 (4 timesteps each)

    xT_e = nc.declare_dram_parameter("xT", [I_DIM, SB], F32R, isOutput=False)
    w_e = {}
    for d in ("f", "b"):
        w_e["wiT_" + d] = nc.declare_dram_parameter("wiT_" + d, [I_DIM, GS], F32R, isOutput=False)
        w_e["whT_" + d] = nc.declare_dram_parameter("whT_" + d, [H_DIM, GS], F32R, isOutput=False)
        w_e["bias_" + d] = nc.declare_dram_parameter("bias_" + d, [1, GS], F32R, isOutput=False)
    id32_e = nc.declare_dram_parameter("id32", [B, B], F32R, isOutput=False)
    id32f_e = nc.declare_dram_parameter("id32f", [B, B], F32, isOutput=False)
    ones_e = nc.declare_dram_parameter("ones128", [1, P], F32R, isOutput=False)
    out_e = nc.declare_dram_parameter("out", [S, B2, P], F32, isOutput=True)

    xp_d = nc.dram_tensor("xp_scratch", [2, S, B, GS], F32R)

    with tile.TileContext(nc) as tc:
        with (
            tc.tile_pool(name="const", bufs=1) as constp,
            tc.tile_pool(name="xsb", bufs=2) as xsbp,
            tc.tile_pool(name="xpt_st", bufs=2) as xpst,
            tc.tile_pool(name="psumB", bufs=2, space="PSUM") as psumB,
            tc.tile_pool(name="psumCf", bufs=2, space="PSUM") as psumCf,
            tc.tile_pool(name="psumCb", bufs=2, space="PSUM") as psumCb,
            tc.tile_pool(name="psumT", bufs=1, space="PSUM") as psumT,
            tc.tile_pool(name="state", bufs=1) as statep,
            tc.tile_pool(name="step", bufs=3) as stepp,
            tc.tile_pool(name="hcomm", bufs=2) as hcommp,
            tc.tile_pool(name="dram", bufs=2, space="DRAM") as dramp,
        ):
            # ---- constants / weights in SBUF ----
            id32 = constp.tile([B, B], F32R, tag="id32", name="id32")
            nc.sync.dma_start(id32[:], id32_e[:])
            id32f = constp.tile([B, B], F32, tag="id32f", name="id32f")
            nc.sync.dma_start(id32f[:], id32f_e[:])
            ones128 = constp.tile([1, P], F32R, tag="ones", name="ones")
            nc.sync.dma_start(ones128[:], ones_e[:])
            wiT = {}
            whT = {}
            biasT = {}
            for d in ("f", "b"):
                wiT[d] = constp.tile([P, KCH, GS], F32R, tag="wiT" + d, name="wiT" + d)
                nc.sync.dma_start(
                    wiT[d][:],
                    w_e["wiT_" + d][:].rearrange("(k p) g -> p k g", p=P),
                )
                whT[d] = constp.tile([P, KCH, GS], F32R, tag="whT" + d, name="whT" + d)
                nc.sync.dma_start(
                    whT[d][:],
                    w_e["whT_" + d][:].rearrange("(k p) g -> p k g", p=P),
                )
                biasT[d] = constp.tile([1, GS], F32R, tag="bias" + d, name="bias" + d)
                nc.sync.dma_start(biasT[d][:], w_e["bias_" + d][:])

            # ---- x-projection chunk: 4 timesteps x one dir -> xp_d ----
            xp_store = {}

            def xproj_chunk(d, m):
                di = 0 if d == "f" else 1
                xsb = xsbp.tile([P, KCH, P], F32R, tag="xsb", name="xsb")
                nc.sync.dma_start(
                    xsb[:],
                    xT_e[:, m * P:(m + 1) * P].rearrange("(k p) c -> p k c", p=P),
                )
                ps = psumB.tile([P, GS], F32, tag="psB", name="psB")
                nc.tensor.matmul(ps[:], ones128[:], biasT[d][:],
                                 start=True, stop=False)
                for k in range(KCH):
                    nc.tensor.matmul(ps[:], xsb[:, k, :], wiT[d][:, k, :],
                                     start=False, stop=(k == KCH - 1))
                xpt = xpst.tile([P, GS], F32R, tag="xpt", name="xpt")
                nc.vector.tensor_copy(xpt[:], ps[:])
                st = nc.sync.dma_start(
                    xp_d[di, 4 * m:4 * m + 4].rearrange("s b g -> (s b) g"),
                    xpt[:],
                )
                xp_store[(di, m)] = st

            # ---- recurrence ----
            c_state = {d: statep.tile([B, P], F32, tag="c" + d, name="c" + d)
                       for d in ("f", "b")}
            for d in ("f", "b"):
                nc.vector.memset(c_state[d][:], 0.0)

            hT_prev = {"f": None, "b": None}
            psum_pool = {"f": psumCf, "b": psumCb}

            def step(t, xproj_filler):
                spos = {"f": t, "b": S - 1 - t}
                xpt = {}
                ps = {}
                acts = {}
                # -- loads + injects + recurrent matmuls, f then b --
                for d in ("f", "b"):
                    di = 0 if d == "f" else 1
                    xpt[d] = stepp.tile([B, GS], F32R, tag="xp" + d, name="xp" + d)
                    ld = nc.sync.dma_start(xpt[d][:], xp_d[di, spos[d]])
                    if interleave_xproj:
                        add_dep_helper(ld.ins, xp_store[(di, spos[d] // 4)].ins,
                                       sync=True, reason="xp ready")
                    ps[d] = psum_pool[d].tile([B, GS], F32, tag="ps" + d,
                                              name="ps" + d)
                    nc.tensor.matmul(ps[d][:], id32[:], xpt[d][:],
                                     start=True, stop=(t == 0))
                for d in ("f", "b"):
                    if t > 0:
                        hT = hT_prev[d]
                        for k in range(KCH):
                            nc.tensor.matmul(ps[d][:], hT[:, k, :],
                                             whT[d][:, k, :],
                                             start=False, stop=(k == KCH - 1))
                # -- PE filler: emit the xproj chunk between mms and tps --
                if xproj_filler is not None:
                    xproj_chunk(*xproj_filler)
                # -- activations, staggered so dir b is not blocked --
                for d in ("f", "b"):
                    acts[d] = stepp.tile([B, GS], F32, tag="acts" + d,
                                         name="acts" + d)
                    # gate order [i|f|g|o]
                    nc.scalar.activation(acts[d][:, 2 * P:3 * P],
                                         ps[d][:, 2 * P:3 * P], TANH)
                    nc.scalar.activation(acts[d][:, 0:2 * P],
                                         ps[d][:, 0:2 * P], SIG)
                u = {}
                v = {}
                for d in ("f", "b"):
                    u[d] = stepp.tile([B, P], F32, tag="u" + d, name="u" + d)
                    nc.vector.tensor_mul(u[d][:], acts[d][:, 0:P],
                                         acts[d][:, 2 * P:3 * P])
                    v[d] = stepp.tile([B, P], F32, tag="v" + d, name="v" + d)
                    nc.vector.tensor_mul(v[d][:], acts[d][:, P:2 * P],
                                         c_state[d][:])
                    nc.vector.tensor_add(c_state[d][:], u[d][:], v[d][:])
                h = {}
                for d in ("f", "b"):
                    nc.scalar.activation(acts[d][:, 3 * P:4 * P],
                                         ps[d][:, 3 * P:4 * P], SIG)
                    tc_t = stepp.tile([B, P], F32, tag="tc" + d, name="tc" + d)
                    nc.scalar.activation(tc_t[:], c_state[d][:], TANH)
                    h[d] = stepp.tile([B, P], F32, tag="h" + d, name="h" + d)
                    nc.vector.tensor_mul(h[d][:], acts[d][:, 3 * P:4 * P],
                                         tc_t[:])
                    row0 = 0 if d == "f" else B
                    nc.sync.dma_start(out_e[t, row0:row0 + B], h[d][:])
                if t == S - 1:
                    return
                # -- transposes AFTER both matmul groups (PE queue order) --
                for d in ("f", "b"):
                    tp = psumT.tile([P, B], F32, tag="tp" + d, name="tp" + d)
                    nc.tensor.transpose(tp[:], h[d][:], id32f[:])
                    hT_self = hcommp.tile([P, B], F32R, tag="hs" + d,
                                          name="hs" + d)
                    nc.vector.tensor_copy(hT_self[:], tp[:])
                    cc_in = dramp.tile([P, B], F32R, tag="ci" + d,
                                       name="ci" + d)
                    nc.sync.dma_start(cc_in[:], hT_self[:])
                    cc_out = dramp.tile([NCORES * P, B], F32R, tag="co" + d,
                                        name="co" + d)
                    nc.gpsimd.collective_compute(
                        "AllGather",
                        mybir.AluOpType.bypass,
                        ins=[cc_in[:].opt()],
                        outs=[cc_out[:].opt()],
                        replica_groups=[list(range(NCORES))],
                    )
                    hT = hcommp.tile([P, KCH, B], F32R, tag="hr" + d,
                                     name="hr" + d)
                    nc.sync.dma_start(
                        hT[:],
                        cc_out[:].rearrange("(k p) b -> p k b", p=P),
                    )
                    hT_prev[d] = hT

            if interleave_xproj:
                # prologue: the chunks the first steps consume
                for m in (0, 1):
                    xproj_chunk("f", m)
                for m in (MCH - 1, MCH - 2):
                    xproj_chunk("b", m)
                for t in range(S):
                    filler = None
                    if t % 4 == 0 and t // 4 + 2 < MCH:
                        filler = ("f", t // 4 + 2)
                    elif t % 4 == 2 and MCH - 3 - t // 4 >= 0:
                        filler = ("b", MCH - 3 - t // 4)
                    step(t, filler)
            else:
                for m in range(MCH):
                    xproj_chunk("f", m)
                    xproj_chunk("b", m)
                for t in range(S):
                    step(t, None)

    return nc


def fix_drain_waits(nc):
    """This walrus build allows only 1 sync-wait per instruction (2 on
    EventSemaphore).  Move excess waits onto EventSemaphore insts placed
    immediately before the instruction on the same engine."""
    ctr = 0
    for fn in nc.m.functions:
        for bb in fn.blocks:
            insts = list(bb.instructions)
            new = []
            changed = False
            for ins in insts:
                si = ins.sync_info
                if (
                    not isinstance(ins, mybir.InstEventSemaphore)
                    and si is not None
                    and len(si.on_wait) > 1
                ):
                    waits = list(si.on_wait)
                    keep, extra = waits[:1], waits[1:]
                    for i in range(0, len(extra), 2):
                        w = mybir.InstEventSemaphore(
                            name=f"I-dwfix-{ctr}",
                            engine=ins.engine,
                            ins=[],
                            outs=[],
                            sync_info=mybir.SyncInfo(
                                on_wait=extra[i : i + 2], on_update=[]
                            ),
                        )
                        ctr += 1
                        new.append(w)
                    ins.sync_info = mybir.SyncInfo(
                        on_wait=keep, on_update=list(si.on_update)
                    )
                    changed = True
                new.append(ins)
            if changed:
                try:
                    bb.instructions = new
                except Exception:
                    bb.instructions.clear()
                    bb.instructions.extend(new)


def kernel(x, W_ii, W_hi, b_i, W_ii_reverse, W_hi_reverse, b_i_reverse):
    """Full inputs in, full (B, S, 2H) output out."""
    import os

    global LAST_EXEC_NS
    import concourse.bass_utils as bu

    bu.upload_artifacts = lambda tmpdir: "local://" + tmpdir
    from concourse.bass_utils import run_bass_kernel_spmd

    S = S_FIXED
    trace = os.environ.get("TRNLSTM_TRACE", "0") == "1"
    interleave = os.environ.get("TRNLSTM_INTERLEAVE", "1") == "1"

    nc = build_kernel(S, interleave_xproj=interleave)
    nc.compile()
    fix_drain_waits(nc)
    in_maps = host_prep(x, W_ii, W_hi, b_i,
                        W_ii_reverse, W_hi_reverse, b_i_reverse, S)
    res = run_bass_kernel_spmd(nc, in_maps, list(range(NCORES)), trace=trace)
    LAST_EXEC_NS = res.exec_time_ns
    return host_assemble(res.results, S)
